# revision 26
# baseline (speedup 1.0000x reference)
"""AnchorFreeLoss on 8 TRN2 NeuronCores (v3).

Strategy (data-parallel over batch, 2 images/core):
- Host prep (tiny [B,M] per-object math, like the qgrid/NHWC transposes):
  per-object centers/radii -> packed block-diagonal bf16 coefficient
  matrix wt128 (3-way bf16 split of fp32 coefficients, quadrant-aligned),
  gather offsets, box targets, class one-hots, and scatter dedup flags
  (last valid object per cell wins, matching XLA scatter semantics).
- Device heatmap target: logG[pix, m] = -dist^2/(2*sigma^2) is affine in
  q(pix) = [x^2+y^2, x, y, 1]: ONE bf16 matmul per 512-pixel group
  (K=128 packs 4 pixel-chunks x 15 live rows) into two 4-bank PSUM slabs;
  DVE max-reduces a whole 2048-col slab per instruction. Cutoff dist<=2r
  equals logG >= -8 exactly (sigma = r/2).
- Focal transcendentals on the Scalar engine (incl. per-slab exp of the
  log-heatmap); focal algebra + reductions on DVE.
- Box/class losses only touch object-center cells: GPSIMD indirect-DMA
  gathers from host-transposed [B*H*W, C] tables.
- No collective: GPSIMD cross-lane-reduces the per-partition partial
  sums; each core DMAs out 8 floats; host combines (the unshard step).
"""

import sys
from contextlib import ExitStack

import numpy as np

if "/opt/trn_rl_repo" not in sys.path:
    sys.path.insert(0, "/opt/trn_rl_repo")

import ml_dtypes
from concourse import bass, mybir
from concourse.bass_utils import run_bass_kernel_spmd

F32 = mybir.dt.float32
BF16 = mybir.dt.bfloat16
I32 = mybir.dt.int32
ALU = mybir.AluOpType
ACT = mybir.ActivationFunctionType
AXX = mybir.AxisListType.X

B, M, H, W = 16, 64, 160, 160
NC = 8
BPC = B // NC          # 2 images per core
PIX = H * W            # 25600
NCLS = 43
EPS = 1e-7
THR = -8.0             # log-domain cutoff (= dist <= 2r since sigma = r/2)
NGRP = PIX // 512      # 50 groups of 512 pixels
NSLAB = (NGRP + 3) // 4  # 13 slabs (last one half-size)


def _build(debug=False):
    nc = bass.Bass()

    qg_d = nc.declare_dram_parameter("qg", [60, NGRP * 128], BF16, isOutput=False)
    wt_d = nc.declare_dram_parameter("wt", [60, 512], BF16, isOutput=False)
    hm_d = nc.declare_dram_parameter("hm2", [128, 400], F32, isOutput=False)
    pb_d = nc.declare_dram_parameter("pbt", [BPC * PIX, 4], F32, isOutput=False)
    pc_d = nc.declare_dram_parameter("pct", [BPC * PIX, NCLS], F32, isOutput=False)
    cg_d = nc.declare_dram_parameter("cellg", [128, 1], I32, isOutput=False)
    tb_d = nc.declare_dram_parameter("tboxd", [128, 4], F32, isOutput=False)
    oh_d = nc.declare_dram_parameter("onehot", [128, NCLS], F32, isOutput=False)
    pi_d = nc.declare_dram_parameter("parti", [128, 8], F32, isOutput=False)
    cv_d = nc.declare_dram_parameter("cvec", [128, 8], F32, isOutput=False)
    out_d = nc.declare_dram_parameter("out", [128, 8], F32, isOutput=True)
    dbg = {}
    if debug:
        for nm, shp, dt in [("d_partials", [128, 8], F32),
                            ("d_hmL", [128, 400], F32),
                            ("d_sc", [128, 16], F32),
                            ("d_gb", [128, 4], F32),
                            ("d_gbn", [128, 4], F32),
                            ("d_tbox", [128, 4], F32)]:
            dbg[nm] = nc.declare_dram_parameter(nm, shp, dt, isOutput=True)

    es = ExitStack()
    dS = es.enter_context(nc.semaphore("dS"))        # small input dmas
    dH = es.enter_context(nc.semaphore("dH"))        # hm2
    dQ = es.enter_context(nc.semaphore("dQ"))        # qgrid first chunk
    dQ2 = es.enter_context(nc.semaphore("dQ2"))      # qgrid chunk 2
    dQ3 = es.enter_context(nc.semaphore("dQ3"))      # qgrid chunk 3
    dW = es.enter_context(nc.semaphore("dW"))        # wt128
    dC = es.enter_context(nc.semaphore("dC"))        # cellg
    dO = es.enter_context(nc.semaphore("dO"))        # output
    va = es.enter_context(nc.semaphore("va"))        # vector -> scalar
    av = es.enter_context(nc.semaphore("av"))        # scalar -> vector
    g_s = es.enter_context(nc.semaphore("g_s"))      # gathers done
    pe_s = es.enter_context(nc.semaphore("pe_s"))    # matmul per group
    dv_s = es.enter_context(nc.semaphore("dv_s"))    # slab reduce done
    pt_s = es.enter_context(nc.semaphore("pt_s"))    # partials ready
    pp_s = es.enter_context(nc.semaphore("pp_s"))    # psp matmul done
    cp_s = es.enter_context(nc.semaphore("cp_s"))    # scalar psum->sbuf copies
    pv_s = es.enter_context(nc.semaphore("pv_s"))    # pvec ready

    sQ = es.enter_context(nc.sbuf_tensor("sQ", [60, NGRP * 128], BF16))
    wt128 = es.enter_context(nc.sbuf_tensor("wt128", [60, 512], BF16))
    cvec = es.enter_context(nc.sbuf_tensor("cvec_s", [128, 8], F32))
    sci = es.enter_context(nc.sbuf_tensor("sci", [128, 1], I32))
    hmP = es.enter_context(nc.sbuf_tensor("hmP", [128, 400], F32))
    lnp = es.enter_context(nc.sbuf_tensor("lnp", [128, 400], F32))
    ln1p = es.enter_context(nc.sbuf_tensor("ln1p", [128, 400], F32))
    p2 = es.enter_context(nc.sbuf_tensor("p2", [128, 400], F32))
    q2 = es.enter_context(nc.sbuf_tensor("q2", [128, 400], F32))
    texp = es.enter_context(nc.sbuf_tensor("texp", [128, 400], F32))
    hmL = es.enter_context(nc.sbuf_tensor("hmL", [128, 400], F32))
    sbf = [es.enter_context(nc.sbuf_tensor(f"sbf{i}", [128, 2048], F32))
           for i in range(2)]
    tbox = es.enter_context(nc.sbuf_tensor("tbox", [128, 4], F32))
    gb = es.enter_context(nc.sbuf_tensor("gb", [128, 4], F32))
    gbn = es.enter_context(nc.sbuf_tensor("gbn", [128, 4], F32))
    gc = es.enter_context(nc.sbuf_tensor("gc", [128, NCLS], F32))
    gcp = es.enter_context(nc.sbuf_tensor("gcp", [128, NCLS], F32))
    junk43 = es.enter_context(nc.sbuf_tensor("junk43", [128, NCLS], F32))
    jb43 = es.enter_context(nc.sbuf_tensor("jb43", [128, NCLS], F32))
    onehot = es.enter_context(nc.sbuf_tensor("onehot_s", [128, NCLS], F32))
    parti = es.enter_context(nc.sbuf_tensor("parti_s", [128, 8], F32))
    sc = es.enter_context(nc.sbuf_tensor("sc", [128, 16], F32))
    pv = es.enter_context(nc.sbuf_tensor("pv", [1, 8], F32))

    psA = es.enter_context(nc.psum_tensor("psA", [128, 2048], F32))
    psB = es.enter_context(nc.psum_tensor("psB", [128, 2048], F32))

    with es:
        names = ["l1r", "negrow", "plab", "lnpl", "ln1pl", "p2pl", "q2pl",
                 "cva", "sent"]
        col = {n: sc[:, i: i + 1] for i, n in enumerate(names)}
        nc.const_aps.aps[(F32, 0.0)] = cvec[:, 0:1]
        nc.const_aps.aps[(F32, 1.0)] = cvec[:, 1:2]
        ones = cvec[:, 1:2]
        kept = parti[:, 2:3]
        keep2 = parti[:, 5:6]
        cellg = sci[:, 0:1]

        def slab_ngroups(k):
            return min(4, NGRP - 4 * k)

        def slab_in(k):
            pst = psA if k % 2 == 0 else psB
            return pst[:, 0: 512 * slab_ngroups(k)]

        def slab_out(k):
            return hmL[:, 32 * k: 32 * k + 8 * slab_ngroups(k)]

        with nc.Block() as block:

            @block.sync
            def _(sync):
                # critical first: first 8 matmul groups, weights, pred heatmap
                sync.dma_start(out=sQ[:, 0:1024], in_=qg_d[:, 0:1024]).then_inc(dQ, 16)
                sync.dma_start(out=wt128[:, :], in_=wt_d[:, :]).then_inc(dW, 16)
                sync.dma_start(out=hmP[:, :], in_=hm_d[:, :]).then_inc(dH, 16)
                sync.dma_start(out=cvec[:, :], in_=cv_d[:, :]).then_inc(dH, 16)
                sync.dma_start(out=sci[:, :], in_=cg_d[:, :]).then_inc(dC, 16)
                # hold the rest back so the criticals get full DMA bandwidth
                sync.wait_ge(dQ, 16)
                sync.wait_ge(dH, 32)
                sync.dma_start(out=parti[:, :], in_=pi_d[:, :]).then_inc(dS, 16)
                sync.dma_start(out=tbox[:, :], in_=tb_d[:, :]).then_inc(dS, 16)
                sync.dma_start(out=sQ[:, 1024:3072], in_=qg_d[:, 1024:3072]).then_inc(dQ2, 16)
                sync.dma_start(out=sQ[:, 3072:], in_=qg_d[:, 3072:]).then_inc(dQ3, 16)
                sync.dma_start(out=onehot[:, :], in_=oh_d[:, :]).then_inc(dS, 16)
                sync.wait_ge(pt_s, 1)
                sync.dma_start(out=out_d[:, :], in_=parti[:, :]).then_inc(dO, 16)
                ndO = 16
                if debug:
                    for nm, t in [("d_partials", parti), ("d_hmL", hmL),
                                  ("d_sc", sc), ("d_gb", gb),
                                  ("d_gbn", gbn), ("d_tbox", tbox)]:
                        sync.dma_start(out=dbg[nm][:, :], in_=t[:, :]).then_inc(dO, 16)
                        ndO += 16
                sync.wait_ge(dO, ndO)

            @block.gpsimd
            def _(gpsimd):
                gpsimd.wait_ge(dC, 16)
                gpsimd.indirect_dma_start(
                    out=gb[:, :], out_offset=None,
                    in_=pb_d[:, :],
                    in_offset=bass.IndirectOffsetOnAxis(ap=cellg, axis=0),
                ).then_inc(g_s, 16)
                gpsimd.indirect_dma_start(
                    out=gc[:, :], out_offset=None,
                    in_=pc_d[:, :],
                    in_offset=bass.IndirectOffsetOnAxis(ap=cellg, axis=0),
                ).then_inc(g_s, 16)


            @block.tensor
            def _(tensor):
                tensor.wait_ge(dW, 16)
                tensor.wait_ge(dQ, 16)
                for g in range(NGRP):
                    if g == 8:
                        tensor.wait_ge(dQ2, 16)
                    if g == 24:
                        tensor.wait_ge(dQ3, 16)
                    pst = psA if (g // 4) % 2 == 0 else psB
                    if g >= 8:
                        tensor.wait_ge(dv_s, g // 4 - 1)
                    tensor.matmul(
                        pst[:, 512 * (g % 4): 512 * (g % 4 + 1)],
                        sQ[:, g * 128: (g + 1) * 128],
                        wt128[:, :],
                        start=True, stop=True, skip_group_check=True,
                    ).then_inc(pe_s, 1)

            @block.scalar
            def _(scalar):
                OFF = [2, 4, 6, 8, 10]

                def off_copies(k, idx):
                    # stage slab k's four psum banks into sbuf for a faster
                    # DVE reduce; sbuf buffer reuse gated by the reduce 2 back
                    if idx >= 2:
                        scalar.wait_ge(dv_s, OFF[idx - 2] + 1)
                    pst = psA if k % 2 == 0 else psB
                    for j in range(4):
                        scalar.wait_ge(pe_s, 4 * k + j + 1)
                        scalar.activation(sbf[idx % 2][:, 512 * j: 512 * (j + 1)],
                                          pst[:, 512 * j: 512 * (j + 1)],
                                          ACT.Copy).then_inc(cp_s, 1)

                off_copies(2, 0)
                # pred-heatmap transcendentals (read clipped hmP only)
                scalar.wait_ge(va, 1)
                scalar.activation(lnp[:, :], hmP[:, :], ACT.Ln)
                scalar.activation(ln1p[:, :], hmP[:, :], ACT.Ln, bias=1.0, scale=-1.0)
                scalar.activation(p2[:, :], hmP[:, :], ACT.Square)
                scalar.activation(q2[:, :], hmP[:, :], ACT.Square, bias=1.0, scale=-1.0)
                scalar.drain().then_inc(av, 1)                      # av1
                # cls: p = sigmoid(x), softplus(x) = -ln(1-p), p^2 via tables
                scalar.wait_ge(g_s, 32)
                scalar.activation(gcp[:, :], gc[:, :], ACT.Sigmoid)
                scalar.activation(junk43[:, :], gcp[:, :], ACT.Ln, bias=1.0, scale=-1.0)
                scalar.activation(gc[:, :], gcp[:, :], ACT.Square)
                scalar.drain().then_inc(av, 1)                      # av2
                off_copies(4, 1)
                off_copies(6, 2)
                scalar.wait_ge(va, 2)
                scalar.activation(col["lnpl"], col["plab"], ACT.Ln)
                scalar.activation(col["ln1pl"], col["plab"], ACT.Ln, bias=1.0, scale=-1.0)
                scalar.activation(col["p2pl"], col["plab"], ACT.Square)
                scalar.activation(col["q2pl"], col["plab"], ACT.Square, bias=1.0, scale=-1.0)
                scalar.drain().then_inc(av, 1)                      # av3
                off_copies(8, 3)
                off_copies(10, 4)
                # per-slab exp of the log-heatmap target
                for k in range(NSLAB):
                    scalar.wait_ge(dv_s, min(k + 2, NSLAB))
                    n8 = 8 * slab_ngroups(k)
                    scalar.activation(texp[:, 32 * k: 32 * k + n8],
                                      hmL[:, 32 * k: 32 * k + n8], ACT.Exp)
                scalar.drain().then_inc(av, 1)                      # av4

            @block.vector
            def _(v):
                ts, stt = v.tensor_scalar, v.scalar_tensor_tensor

                def slot(*thunks):
                    for t in thunks:
                        t()
                    v.drain()

                # heatmap slab reduces + interleaved one-shot work
                for k in range(NSLAB):
                    if k == 0:
                        v.wait_ge(pe_s, 2)
                        v.tensor_reduce(
                            out=hmL[:, 0:16],
                            in_=psA[:, 0:1024].rearrange("p (G m) -> p G m", G=16),
                            op=ALU.max, axis=AXX)
                        v.wait_ge(pe_s, 4)
                        v.tensor_reduce(
                            out=hmL[:, 16:32],
                            in_=psA[:, 1024:2048].rearrange("p (G m) -> p G m", G=16),
                            op=ALU.max, axis=AXX,
                        ).then_inc(dv_s, 1)
                        continue
                    OFFV = [2, 4, 6, 8, 10]
                    if k in OFFV:
                        idx = OFFV.index(k)
                        v.wait_ge(cp_s, 4 * (idx + 1))
                        v.tensor_reduce(
                            out=slab_out(k),
                            in_=sbf[idx % 2][:, :].rearrange("p (G m) -> p G m", G=32),
                            op=ALU.max, axis=AXX,
                        ).then_inc(dv_s, 1)
                    else:
                        v.wait_ge(pe_s, min(4 * k + 4, NGRP))
                        v.tensor_reduce(
                            out=slab_out(k),
                            in_=slab_in(k).rearrange("p (G m) -> p G m",
                                                     G=8 * slab_ngroups(k)),
                            op=ALU.max, axis=AXX,
                        ).then_inc(dv_s, 1)
                    if k == 1:
                        # clip predicted heatmap + box l1 (independent, share slots)
                        v.wait_ge(dH, 32)
                        v.wait_ge(g_s, 32)
                        v.wait_ge(dS, 32)
                        slot(lambda: ts(hmP[:, :], hmP[:, :], EPS, 1.0 - EPS, op0=ALU.max, op1=ALU.min),
                             lambda: v.tensor_sub(gbn[:, :], gb[:, :], tbox[:, :]))
                        v.sem_inc(va, 1)                            # va1
                        slot(lambda: stt(gbn[:, :], gbn[:, :], -1.0, gbn[:, :], op0=ALU.mult, op1=ALU.max))
                        slot(lambda: v.tensor_reduce(out=col["l1r"], in_=gbn[:, :], op=ALU.add, axis=AXX))
                        slot(lambda: v.tensor_mul(parti[:, 3:4], col["l1r"], kept))
                    if k == 3:
                        # focal A/B coefficient maps
                        v.wait_ge(av, 1)
                        slot(lambda: stt(lnp[:, :], q2[:, :], -0.25, lnp[:, :], op0=ALU.mult, op1=ALU.mult),
                             lambda: stt(ln1p[:, :], p2[:, :], 0.75, ln1p[:, :], op0=ALU.mult, op1=ALU.mult))
                    if k == 5:
                        v.wait_ge(av, 2)
                        slot(lambda: stt(junk43[:, :], gc[:, :], -0.75, junk43[:, :],
                                         op0=ALU.mult, op1=ALU.mult, accum_out=col["negrow"]),
                             lambda: v.tensor_mul(jb43[:, :], gcp[:, :], onehot[:, :]))
                        slot(lambda: v.tensor_mul(parti[:, 4:5], col["negrow"], kept),
                             lambda: v.tensor_reduce(out=col["plab"], in_=jb43[:, :], op=ALU.add, axis=AXX))
                        v.sem_inc(va, 1)                            # va2 (plab ready)
                    if k == 8:
                        v.wait_ge(av, 3)
                        slot(lambda: stt(col["cva"], col["q2pl"], -0.25, col["lnpl"], op0=ALU.mult, op1=ALU.mult),
                             lambda: stt(col["sent"], col["p2pl"], -0.75, col["ln1pl"], op0=ALU.mult, op1=ALU.mult))
                        slot(lambda: v.tensor_sub(col["cva"], col["cva"], col["sent"]))
                        slot(lambda: v.tensor_mul(parti[:, 6:7], col["cva"], keep2))
                # ---- heat focal tail (cutoff mask dropped: exp(logG) <= e^-8
                # below threshold, which perturbs only (1-t) by <=3.4e-4) ----
                v.drain()
                v.wait_ge(av, 4)
                slot(lambda: v.tensor_mul(lnp[:, :], lnp[:, :], texp[:, :]),
                     lambda: stt(ln1p[:, :], texp[:, :], 1.0, ln1p[:, :], op0=ALU.subtract, op1=ALU.mult))
                slot(lambda: ts(q2[:, :], texp[:, :], 0.5, None, op0=ALU.is_gt))
                slot(lambda: v.tensor_reduce(out=parti[:, 0:1], in_=q2[:, :], op=ALU.add, axis=AXX),
                     lambda: v.tensor_sub(lnp[:, :], lnp[:, :], ln1p[:, :]))
                slot(lambda: v.tensor_mul(lnp[:, :], lnp[:, :], q2[:, :]))
                slot(lambda: stt(lnp[:, :], lnp[:, :], 1.0, ln1p[:, :],
                                 op0=ALU.mult, op1=ALU.add, accum_out=parti[:, 1:2]))
                v.sem_inc(pt_s, 1)

    return nc


_CACHE = {}


def _consts():
    j = np.arange(PIX)
    x = (j % W).astype(np.float32)
    y = (j // W).astype(np.float32)
    q1 = x * x + y * y
    q1hi = q1.astype(ml_dtypes.bfloat16).astype(np.float32)
    q1lo = q1 - q1hi
    onesv = np.ones_like(x)
    q5 = np.stack([q1hi, q1lo, x, y, onesv])           # [5, PIX] all bf16-exact
    # qg[15 c + 5 s + r, 128 g + p] = q5[r, 512 g + 128 c + p]
    q5r = q5.reshape(5, NGRP, 4, 128)                  # [r, g, c, p]
    qg = np.zeros((60, NGRP * 128), np.float32)
    for c in range(4):
        for s in range(3):
            qg[15 * c + 5 * s: 15 * c + 5 * s + 5, :] = (
                q5r[:, :, c, :].reshape(5, NGRP * 128))
    qg = qg.astype(ml_dtypes.bfloat16)
    cvec = np.zeros((128, 8), dtype=np.float32)
    cvec[:, 1] = 1.0
    return qg, cvec


def _last_wins_kept(keys, valid):
    """kept[i] = valid[i] and no valid j>i with keys[j]==keys[i]."""
    n = len(keys)
    kept = np.zeros(n, bool)
    seen = set()
    for i in range(n - 1, -1, -1):
        if valid[i] and keys[i] not in seen:
            kept[i] = True
            seen.add(keys[i])
    return kept


def _stage_a(bboxes, labels):
    """Per-core-chunk object prep: returns wt128, cellg, tbox, onehot, parti."""
    f32 = np.float32
    bb = bboxes.reshape(128, 4).astype(f32)
    lab = labels.reshape(128).astype(np.int64)
    x1, y1, x2, y2 = bb[:, 0], bb[:, 1], bb[:, 2], bb[:, 3]
    cx = (x1 + x2) / f32(2.0)
    cy = (y1 + y2) / f32(2.0)
    bw = x2 - x1
    bh = y2 - y1
    valid = (lab >= 0) & (bb.sum(1) > 0) & (bw > 0) & (bh > 0)
    gx = np.clip((cx / f32(4.0)).astype(np.int32), 0, W - 1)
    gy = np.clip((cy / f32(4.0)).astype(np.int32), 0, H - 1)
    r = np.maximum(np.sqrt(np.maximum(bw * bh, f32(0.0))) / f32(4.0), f32(2.0)).astype(np.int32).astype(f32)
    nscv = f32(-2.0) / (r * r)
    gxf = gx.astype(f32)
    gyf = gy.astype(f32)
    w1 = np.where(valid, nscv, f32(0))
    w2 = np.where(valid, f32(-2.0) * nscv * gxf, f32(0))
    w3 = np.where(valid, f32(-2.0) * nscv * gyf, f32(0))
    w4 = np.where(valid, nscv * (gxf * gxf + gyf * gyf), f32(-1e30))
    Wm = np.stack([w1, w1, w2, w3, w4]).astype(f32)    # [5, 128]
    # 3-way bf16 split
    a_ = Wm.astype(ml_dtypes.bfloat16).astype(f32)
    r1_ = Wm - a_
    b_ = r1_.astype(ml_dtypes.bfloat16).astype(f32)
    r2_ = r1_ - b_
    c_ = r2_.astype(ml_dtypes.bfloat16).astype(f32)
    w15 = np.concatenate([a_, b_, c_], axis=0)         # [15, 128]
    wt128 = np.zeros((60, 512), np.float32)
    for c in range(4):
        wt128[15 * c: 15 * c + 15, 128 * c: 128 * (c + 1)] = w15
    wt128 = wt128.astype(ml_dtypes.bfloat16)

    img = np.arange(128) // M
    cell = gy.astype(np.int64) * W + gx.astype(np.int64)
    cellg = (cell + img * PIX).astype(np.int32)
    kept = _last_wins_kept(list(cellg), valid)
    labc = np.clip(lab, 0, NCLS - 1)
    key2 = cellg.astype(np.int64) * NCLS + labc
    keep2 = _last_wins_kept(list(key2), valid)

    tbox = np.zeros((128, 4), np.float32)
    tbox[:, 0] = np.where(valid, cx / f32(4.0) - gxf - f32(0.5), f32(0.0))
    tbox[:, 1] = np.where(valid, cy / f32(4.0) - gyf - f32(0.5), f32(0.0))
    tbox[:, 2] = np.where(valid, np.log(np.maximum(bw * f32(0.25) + f32(1e-6), f32(1e-20))).astype(f32), f32(0.0))
    tbox[:, 3] = np.where(valid, np.log(np.maximum(bh * f32(0.25) + f32(1e-6), f32(1e-20))).astype(f32), f32(0.0))
    onehot = (labc[:, None] == np.arange(NCLS)[None, :]).astype(np.float32)
    parti = np.zeros((128, 8), np.float32)
    parti[:, 2] = kept.astype(np.float32)
    parti[:, 5] = keep2.astype(np.float32)
    return wt128, cellg.reshape(128, 1), tbox, onehot, parti


def _prep(pred_heatmap, pred_boxes, pred_classes, bboxes, labels):
    qg, cvec = _consts()
    pbt = np.ascontiguousarray(
        pred_boxes.transpose(0, 2, 3, 1).reshape(B, PIX, 4)).astype(np.float32)
    pct = np.ascontiguousarray(
        pred_classes.transpose(0, 2, 3, 1).reshape(B, PIX, NCLS)).astype(np.float32)
    # hm2[p, 2 f + img] = hm[img, 128 f + p]
    hmr = np.asarray(pred_heatmap, np.float32).reshape(B, 200, 128)
    bbn = np.asarray(bboxes, np.float32)
    labn = np.asarray(labels)
    in_maps = []
    for c in range(NC):
        s = slice(c * BPC, (c + 1) * BPC)
        hm2 = np.ascontiguousarray(hmr[s].transpose(2, 1, 0).reshape(128, 400))
        wt128, cellg, tbox, onehot, parti = _stage_a(bbn[s], labn[s])
        in_maps.append({
            "qg": qg,
            "wt": wt128,
            "hm2": hm2,
            "pbt": pbt[s].reshape(BPC * PIX, 4),
            "pct": pct[s].reshape(BPC * PIX, NCLS),
            "cellg": cellg, "tboxd": tbox, "onehot": onehot,
            "parti": parti, "cvec": cvec,
        })
    return in_maps


def _combine(pvecs):
    P = np.sum(np.stack(pvecs, 0), axis=0, dtype=np.float64).astype(np.float32)
    heat = P[1] / max(P[0], np.float32(1.0))
    num_pos = max(P[2], np.float32(1.0))
    box = P[3] / num_pos if P[2] > 1.0 else np.float32(0.0)
    cls = (P[4] + P[6]) / max(P[5], np.float32(1.0)) if P[2] > 1.0 else np.float32(0.0)
    return np.float32(heat + box + cls)


def kernel(pred_heatmap, pred_boxes, pred_classes, bboxes, labels):
    if "nc" not in _CACHE:
        _CACHE["nc"] = _build()
    nc = _CACHE["nc"]
    in_maps = _prep(pred_heatmap, pred_boxes, pred_classes, bboxes, labels)
    r = run_bass_kernel_spmd(nc, in_maps, list(range(NC)))
    pvecs = [np.asarray(r.results[c]["out"]).reshape(128, 8).sum(axis=0, dtype=np.float64) for c in range(NC)]
    return _combine(pvecs)


if __name__ == "__main__":
    import reference
    inputs = reference.setup_inputs()
    inputs = {k: np.asarray(v) for k, v in inputs.items()}
    out = kernel(**inputs)
    exp = np.asarray(reference.reference(**inputs))
    rel = abs(out - exp) / max(abs(exp), 1e-9)
    print("expected:", exp, "actual:", out, "rel:", rel)


# revision 27
# speedup vs baseline: 1.0845x; 1.0845x over previous
"""AnchorFreeLoss on 8 TRN2 NeuronCores (v3).

Strategy (data-parallel over batch, 2 images/core):
- Host prep (tiny [B,M] per-object math, like the qgrid/NHWC transposes):
  per-object centers/radii -> packed block-diagonal bf16 coefficient
  matrix wt128 (3-way bf16 split of fp32 coefficients, quadrant-aligned),
  gather offsets, box targets, class one-hots, and scatter dedup flags
  (last valid object per cell wins, matching XLA scatter semantics).
- Device heatmap target: logG[pix, m] = -dist^2/(2*sigma^2) is affine in
  q(pix) = [x^2+y^2, x, y, 1]: ONE bf16 matmul per 512-pixel group
  (K=128 packs 4 pixel-chunks x 15 live rows) into two 4-bank PSUM slabs;
  DVE max-reduces a whole 2048-col slab per instruction. Cutoff dist<=2r
  equals logG >= -8 exactly (sigma = r/2).
- Focal transcendentals on the Scalar engine (incl. per-slab exp of the
  log-heatmap); focal algebra + reductions on DVE.
- Box/class losses only touch object-center cells: GPSIMD indirect-DMA
  gathers from host-transposed [B*H*W, C] tables.
- No collective: GPSIMD cross-lane-reduces the per-partition partial
  sums; each core DMAs out 8 floats; host combines (the unshard step).
"""

import sys
from contextlib import ExitStack

import numpy as np

if "/opt/trn_rl_repo" not in sys.path:
    sys.path.insert(0, "/opt/trn_rl_repo")

import ml_dtypes
from concourse import bass, mybir
from concourse.bass_utils import run_bass_kernel_spmd

F32 = mybir.dt.float32
BF16 = mybir.dt.bfloat16
I32 = mybir.dt.int32
ALU = mybir.AluOpType
ACT = mybir.ActivationFunctionType
AXX = mybir.AxisListType.X

B, M, H, W = 16, 64, 160, 160
NC = 8
BPC = B // NC          # 2 images per core
PIX = H * W            # 25600
NCLS = 43
EPS = 1e-7
THR = -8.0             # log-domain cutoff (= dist <= 2r since sigma = r/2)
NGRP = PIX // 512      # 50 groups of 512 pixels
NSLAB = (NGRP + 3) // 4  # 13 slabs (last one half-size)


def _build(debug=False):
    nc = bass.Bass()

    qg_d = nc.declare_dram_parameter("qg", [60, NGRP * 128], BF16, isOutput=False)
    wt_d = nc.declare_dram_parameter("wt", [60, 512], BF16, isOutput=False)
    hm_d = nc.declare_dram_parameter("hm2", [128, 400], F32, isOutput=False)
    pb_d = nc.declare_dram_parameter("pbt", [BPC * PIX, 4], F32, isOutput=False)
    pc_d = nc.declare_dram_parameter("pct", [BPC * PIX, NCLS], F32, isOutput=False)
    cg_d = nc.declare_dram_parameter("cellg", [128, 1], I32, isOutput=False)
    tb_d = nc.declare_dram_parameter("tboxd", [128, 4], F32, isOutput=False)
    oh_d = nc.declare_dram_parameter("onehot", [128, NCLS], F32, isOutput=False)
    pi_d = nc.declare_dram_parameter("parti", [128, 8], F32, isOutput=False)
    cv_d = nc.declare_dram_parameter("cvec", [128, 8], F32, isOutput=False)
    out_d = nc.declare_dram_parameter("out", [128, 8], F32, isOutput=True)
    dbg = {}
    if debug:
        for nm, shp, dt in [("d_partials", [128, 8], F32),
                            ("d_hmL", [128, 400], F32),
                            ("d_sc", [128, 16], F32),
                            ("d_gb", [128, 4], F32),
                            ("d_gbn", [128, 4], F32),
                            ("d_tbox", [128, 4], F32)]:
            dbg[nm] = nc.declare_dram_parameter(nm, shp, dt, isOutput=True)

    es = ExitStack()
    dS = es.enter_context(nc.semaphore("dS"))        # small input dmas
    dH = es.enter_context(nc.semaphore("dH"))        # hm2
    dQ = es.enter_context(nc.semaphore("dQ"))        # qgrid first chunk
    dQ2 = es.enter_context(nc.semaphore("dQ2"))      # qgrid chunk 2
    dQ3 = es.enter_context(nc.semaphore("dQ3"))      # qgrid chunk 3
    dW = es.enter_context(nc.semaphore("dW"))        # wt128
    dC = es.enter_context(nc.semaphore("dC"))        # cellg
    dO = es.enter_context(nc.semaphore("dO"))        # output
    va = es.enter_context(nc.semaphore("va"))        # vector -> scalar
    av = es.enter_context(nc.semaphore("av"))        # scalar -> vector
    g_s = es.enter_context(nc.semaphore("g_s"))      # gathers done
    pe_s = es.enter_context(nc.semaphore("pe_s"))    # matmul per group
    dv_s = es.enter_context(nc.semaphore("dv_s"))    # slab reduce done
    pt_s = es.enter_context(nc.semaphore("pt_s"))    # partials ready
    pp_s = es.enter_context(nc.semaphore("pp_s"))    # psp matmul done
    pv_s = es.enter_context(nc.semaphore("pv_s"))    # pvec ready

    sQ = es.enter_context(nc.sbuf_tensor("sQ", [60, NGRP * 128], BF16))
    wt128 = es.enter_context(nc.sbuf_tensor("wt128", [60, 512], BF16))
    cvec = es.enter_context(nc.sbuf_tensor("cvec_s", [128, 8], F32))
    sci = es.enter_context(nc.sbuf_tensor("sci", [128, 1], I32))
    hmP = es.enter_context(nc.sbuf_tensor("hmP", [128, 400], F32))
    lnp = es.enter_context(nc.sbuf_tensor("lnp", [128, 400], F32))
    ln1p = es.enter_context(nc.sbuf_tensor("ln1p", [128, 400], F32))
    p2 = es.enter_context(nc.sbuf_tensor("p2", [128, 400], F32))
    q2 = es.enter_context(nc.sbuf_tensor("q2", [128, 400], F32))
    texp = es.enter_context(nc.sbuf_tensor("texp", [128, 400], F32))
    hmL = es.enter_context(nc.sbuf_tensor("hmL", [128, 400], F32))
    tbox = es.enter_context(nc.sbuf_tensor("tbox", [128, 4], F32))
    gb = es.enter_context(nc.sbuf_tensor("gb", [128, 4], F32))
    gbn = es.enter_context(nc.sbuf_tensor("gbn", [128, 4], F32))
    gc = es.enter_context(nc.sbuf_tensor("gc", [128, NCLS], F32))
    gcp = es.enter_context(nc.sbuf_tensor("gcp", [128, NCLS], F32))
    junk43 = es.enter_context(nc.sbuf_tensor("junk43", [128, NCLS], F32))
    jb43 = es.enter_context(nc.sbuf_tensor("jb43", [128, NCLS], F32))
    onehot = es.enter_context(nc.sbuf_tensor("onehot_s", [128, NCLS], F32))
    parti = es.enter_context(nc.sbuf_tensor("parti_s", [128, 8], F32))
    sc = es.enter_context(nc.sbuf_tensor("sc", [128, 16], F32))
    pv = es.enter_context(nc.sbuf_tensor("pv", [1, 8], F32))

    psA = es.enter_context(nc.psum_tensor("psA", [128, 2048], F32))
    psB = es.enter_context(nc.psum_tensor("psB", [128, 2048], F32))

    with es:
        names = ["l1r", "negrow", "plab", "lnpl", "ln1pl", "p2pl", "q2pl",
                 "cva", "sent"]
        col = {n: sc[:, i: i + 1] for i, n in enumerate(names)}
        nc.const_aps.aps[(F32, 0.0)] = cvec[:, 0:1]
        nc.const_aps.aps[(F32, 1.0)] = cvec[:, 1:2]
        ones = cvec[:, 1:2]
        kept = parti[:, 2:3]
        keep2 = parti[:, 5:6]
        cellg = sci[:, 0:1]

        def slab_ngroups(k):
            return min(4, NGRP - 4 * k)

        def slab_in(k):
            pst = psA if k % 2 == 0 else psB
            return pst[:, 0: 512 * slab_ngroups(k)]

        def slab_out(k):
            return hmL[:, 32 * k: 32 * k + 8 * slab_ngroups(k)]

        with nc.Block() as block:

            @block.sync
            def _(sync):
                # critical first: first 8 matmul groups, weights, pred heatmap
                sync.dma_start(out=sQ[:, 0:1024], in_=qg_d[:, 0:1024]).then_inc(dQ, 16)
                sync.dma_start(out=wt128[:, :], in_=wt_d[:, :]).then_inc(dW, 16)
                sync.dma_start(out=hmP[:, :], in_=hm_d[:, :]).then_inc(dH, 16)
                sync.dma_start(out=cvec[:, :], in_=cv_d[:, :]).then_inc(dH, 16)
                sync.dma_start(out=sci[:, :], in_=cg_d[:, :]).then_inc(dC, 16)
                # hold the rest back so the criticals get full DMA bandwidth
                sync.wait_ge(dQ, 16)
                sync.wait_ge(dH, 32)
                sync.dma_start(out=parti[:, :], in_=pi_d[:, :]).then_inc(dS, 16)
                sync.dma_start(out=tbox[:, :], in_=tb_d[:, :]).then_inc(dS, 16)
                sync.dma_start(out=sQ[:, 1024:3072], in_=qg_d[:, 1024:3072]).then_inc(dQ2, 16)
                sync.dma_start(out=sQ[:, 3072:], in_=qg_d[:, 3072:]).then_inc(dQ3, 16)
                sync.dma_start(out=onehot[:, :], in_=oh_d[:, :]).then_inc(dS, 16)
                sync.wait_ge(pt_s, 1)
                sync.dma_start(out=out_d[:, :], in_=parti[:, :]).then_inc(dO, 16)
                ndO = 16
                if debug:
                    for nm, t in [("d_partials", parti), ("d_hmL", hmL),
                                  ("d_sc", sc), ("d_gb", gb),
                                  ("d_gbn", gbn), ("d_tbox", tbox)]:
                        sync.dma_start(out=dbg[nm][:, :], in_=t[:, :]).then_inc(dO, 16)
                        ndO += 16
                sync.wait_ge(dO, ndO)

            @block.gpsimd
            def _(gpsimd):
                gpsimd.wait_ge(dC, 16)
                gpsimd.indirect_dma_start(
                    out=gb[:, :], out_offset=None,
                    in_=pb_d[:, :],
                    in_offset=bass.IndirectOffsetOnAxis(ap=cellg, axis=0),
                ).then_inc(g_s, 16)
                gpsimd.indirect_dma_start(
                    out=gc[:, :], out_offset=None,
                    in_=pc_d[:, :],
                    in_offset=bass.IndirectOffsetOnAxis(ap=cellg, axis=0),
                ).then_inc(g_s, 16)


            @block.tensor
            def _(tensor):
                tensor.wait_ge(dW, 16)
                tensor.wait_ge(dQ, 16)
                for g in range(NGRP):
                    if g == 8:
                        tensor.wait_ge(dQ2, 16)
                    if g == 24:
                        tensor.wait_ge(dQ3, 16)
                    pst = psA if (g // 4) % 2 == 0 else psB
                    if g >= 8:
                        tensor.wait_ge(dv_s, g // 4 - 1)
                    tensor.matmul(
                        pst[:, 512 * (g % 4): 512 * (g % 4 + 1)],
                        sQ[:, g * 128: (g + 1) * 128],
                        wt128[:, :],
                        start=True, stop=True, skip_group_check=True,
                    ).then_inc(pe_s, 1)

            @block.scalar
            def _(scalar):
                # pred-heatmap transcendentals (read clipped hmP only)
                scalar.wait_ge(va, 1)
                scalar.activation(lnp[:, :], hmP[:, :], ACT.Ln)
                scalar.activation(ln1p[:, :], hmP[:, :], ACT.Ln, bias=1.0, scale=-1.0)
                scalar.activation(p2[:, :], hmP[:, :], ACT.Square)
                scalar.activation(q2[:, :], hmP[:, :], ACT.Square, bias=1.0, scale=-1.0)
                scalar.drain().then_inc(av, 1)                      # av1
                # cls: p = sigmoid(x), softplus(x) = -ln(1-p), p^2 via tables
                scalar.wait_ge(g_s, 32)
                scalar.activation(gcp[:, :], gc[:, :], ACT.Sigmoid)
                scalar.activation(junk43[:, :], gcp[:, :], ACT.Ln, bias=1.0, scale=-1.0)
                scalar.activation(gc[:, :], gcp[:, :], ACT.Square)
                scalar.drain().then_inc(av, 1)                      # av2
                scalar.wait_ge(va, 2)
                scalar.activation(col["lnpl"], col["plab"], ACT.Ln)
                scalar.activation(col["ln1pl"], col["plab"], ACT.Ln, bias=1.0, scale=-1.0)
                scalar.activation(col["p2pl"], col["plab"], ACT.Square)
                scalar.activation(col["q2pl"], col["plab"], ACT.Square, bias=1.0, scale=-1.0)
                scalar.drain().then_inc(av, 1)                      # av3
                # per-slab exp of the log-heatmap target
                for k in range(NSLAB):
                    scalar.wait_ge(dv_s, min(k + 2, NSLAB))
                    n8 = 8 * slab_ngroups(k)
                    scalar.activation(texp[:, 32 * k: 32 * k + n8],
                                      hmL[:, 32 * k: 32 * k + n8], ACT.Exp)
                scalar.drain().then_inc(av, 1)                      # av4

            @block.vector
            def _(v):
                ts, stt = v.tensor_scalar, v.scalar_tensor_tensor

                def slot(*thunks):
                    for t in thunks:
                        t()
                    v.drain()

                # heatmap slab reduces + interleaved one-shot work
                for k in range(NSLAB):
                    if k == 0:
                        v.wait_ge(pe_s, 2)
                        v.tensor_reduce(
                            out=hmL[:, 0:16],
                            in_=psA[:, 0:1024].rearrange("p (G m) -> p G m", G=16),
                            op=ALU.max, axis=AXX)
                        v.wait_ge(pe_s, 4)
                        v.tensor_reduce(
                            out=hmL[:, 16:32],
                            in_=psA[:, 1024:2048].rearrange("p (G m) -> p G m", G=16),
                            op=ALU.max, axis=AXX,
                        ).then_inc(dv_s, 1)
                        continue
                    v.wait_ge(pe_s, min(4 * k + 4, NGRP))
                    v.tensor_reduce(
                        out=slab_out(k),
                        in_=slab_in(k).rearrange("p (G m) -> p G m",
                                                 G=8 * slab_ngroups(k)),
                        op=ALU.max, axis=AXX,
                    ).then_inc(dv_s, 1)
                    if k == 1:
                        # clip predicted heatmap + box l1 (independent, share slots)
                        v.wait_ge(dH, 32)
                        v.wait_ge(g_s, 32)
                        v.wait_ge(dS, 32)
                        slot(lambda: ts(hmP[:, :], hmP[:, :], EPS, 1.0 - EPS, op0=ALU.max, op1=ALU.min),
                             lambda: v.tensor_sub(gbn[:, :], gb[:, :], tbox[:, :]))
                        v.sem_inc(va, 1)                            # va1
                        slot(lambda: stt(gbn[:, :], gbn[:, :], -1.0, gbn[:, :], op0=ALU.mult, op1=ALU.max))
                        slot(lambda: v.tensor_reduce(out=col["l1r"], in_=gbn[:, :], op=ALU.add, axis=AXX))
                        slot(lambda: v.tensor_mul(parti[:, 3:4], col["l1r"], kept))
                    if k == 3:
                        # focal A/B coefficient maps
                        v.wait_ge(av, 1)
                        slot(lambda: stt(lnp[:, :], q2[:, :], -0.25, lnp[:, :], op0=ALU.mult, op1=ALU.mult),
                             lambda: stt(ln1p[:, :], p2[:, :], 0.75, ln1p[:, :], op0=ALU.mult, op1=ALU.mult))
                    if k == 5:
                        v.wait_ge(av, 2)
                        slot(lambda: stt(junk43[:, :], gc[:, :], -0.75, junk43[:, :],
                                         op0=ALU.mult, op1=ALU.mult, accum_out=col["negrow"]),
                             lambda: v.tensor_mul(jb43[:, :], gcp[:, :], onehot[:, :]))
                        slot(lambda: v.tensor_mul(parti[:, 4:5], col["negrow"], kept),
                             lambda: v.tensor_reduce(out=col["plab"], in_=jb43[:, :], op=ALU.add, axis=AXX))
                        v.sem_inc(va, 1)                            # va2 (plab ready)
                    if k == 8:
                        v.wait_ge(av, 3)
                        slot(lambda: stt(col["cva"], col["q2pl"], -0.25, col["lnpl"], op0=ALU.mult, op1=ALU.mult),
                             lambda: stt(col["sent"], col["p2pl"], -0.75, col["ln1pl"], op0=ALU.mult, op1=ALU.mult))
                        slot(lambda: v.tensor_sub(col["cva"], col["cva"], col["sent"]))
                        slot(lambda: v.tensor_mul(parti[:, 6:7], col["cva"], keep2))
                # ---- heat focal tail (cutoff mask dropped: exp(logG) <= e^-8
                # below threshold, which perturbs only (1-t) by <=3.4e-4) ----
                v.drain()
                v.wait_ge(av, 4)
                slot(lambda: v.tensor_mul(lnp[:, :], lnp[:, :], texp[:, :]),
                     lambda: stt(ln1p[:, :], texp[:, :], 1.0, ln1p[:, :], op0=ALU.subtract, op1=ALU.mult))
                slot(lambda: ts(q2[:, :], texp[:, :], 0.5, None, op0=ALU.is_gt))
                slot(lambda: v.tensor_reduce(out=parti[:, 0:1], in_=q2[:, :], op=ALU.add, axis=AXX),
                     lambda: v.tensor_sub(lnp[:, :], lnp[:, :], ln1p[:, :]))
                slot(lambda: v.tensor_mul(lnp[:, :], lnp[:, :], q2[:, :]))
                slot(lambda: stt(lnp[:, :], lnp[:, :], 1.0, ln1p[:, :],
                                 op0=ALU.mult, op1=ALU.add, accum_out=parti[:, 1:2]))
                v.sem_inc(pt_s, 1)

    return nc


_CACHE = {}


def _consts():
    j = np.arange(PIX)
    x = (j % W).astype(np.float32)
    y = (j // W).astype(np.float32)
    q1 = x * x + y * y
    q1hi = q1.astype(ml_dtypes.bfloat16).astype(np.float32)
    q1lo = q1 - q1hi
    onesv = np.ones_like(x)
    q5 = np.stack([q1hi, q1lo, x, y, onesv])           # [5, PIX] all bf16-exact
    # qg[15 c + 5 s + r, 128 g + p] = q5[r, 512 g + 128 c + p]
    q5r = q5.reshape(5, NGRP, 4, 128)                  # [r, g, c, p]
    qg = np.zeros((60, NGRP * 128), np.float32)
    for c in range(4):
        for s in range(3):
            qg[15 * c + 5 * s: 15 * c + 5 * s + 5, :] = (
                q5r[:, :, c, :].reshape(5, NGRP * 128))
    qg = qg.astype(ml_dtypes.bfloat16)
    cvec = np.zeros((128, 8), dtype=np.float32)
    cvec[:, 1] = 1.0
    return qg, cvec


def _last_wins_kept(keys, valid):
    """kept[i] = valid[i] and no valid j>i with keys[j]==keys[i]."""
    n = len(keys)
    kept = np.zeros(n, bool)
    seen = set()
    for i in range(n - 1, -1, -1):
        if valid[i] and keys[i] not in seen:
            kept[i] = True
            seen.add(keys[i])
    return kept


def _stage_a(bboxes, labels):
    """Per-core-chunk object prep: returns wt128, cellg, tbox, onehot, parti."""
    f32 = np.float32
    bb = bboxes.reshape(128, 4).astype(f32)
    lab = labels.reshape(128).astype(np.int64)
    x1, y1, x2, y2 = bb[:, 0], bb[:, 1], bb[:, 2], bb[:, 3]
    cx = (x1 + x2) / f32(2.0)
    cy = (y1 + y2) / f32(2.0)
    bw = x2 - x1
    bh = y2 - y1
    valid = (lab >= 0) & (bb.sum(1) > 0) & (bw > 0) & (bh > 0)
    gx = np.clip((cx / f32(4.0)).astype(np.int32), 0, W - 1)
    gy = np.clip((cy / f32(4.0)).astype(np.int32), 0, H - 1)
    r = np.maximum(np.sqrt(np.maximum(bw * bh, f32(0.0))) / f32(4.0), f32(2.0)).astype(np.int32).astype(f32)
    nscv = f32(-2.0) / (r * r)
    gxf = gx.astype(f32)
    gyf = gy.astype(f32)
    w1 = np.where(valid, nscv, f32(0))
    w2 = np.where(valid, f32(-2.0) * nscv * gxf, f32(0))
    w3 = np.where(valid, f32(-2.0) * nscv * gyf, f32(0))
    w4 = np.where(valid, nscv * (gxf * gxf + gyf * gyf), f32(-1e30))
    Wm = np.stack([w1, w1, w2, w3, w4]).astype(f32)    # [5, 128]
    # 3-way bf16 split
    a_ = Wm.astype(ml_dtypes.bfloat16).astype(f32)
    r1_ = Wm - a_
    b_ = r1_.astype(ml_dtypes.bfloat16).astype(f32)
    r2_ = r1_ - b_
    c_ = r2_.astype(ml_dtypes.bfloat16).astype(f32)
    w15 = np.concatenate([a_, b_, c_], axis=0)         # [15, 128]
    wt128 = np.zeros((60, 512), np.float32)
    for c in range(4):
        wt128[15 * c: 15 * c + 15, 128 * c: 128 * (c + 1)] = w15
    wt128 = wt128.astype(ml_dtypes.bfloat16)

    img = np.arange(128) // M
    cell = gy.astype(np.int64) * W + gx.astype(np.int64)
    cellg = (cell + img * PIX).astype(np.int32)
    kept = _last_wins_kept(list(cellg), valid)
    labc = np.clip(lab, 0, NCLS - 1)
    key2 = cellg.astype(np.int64) * NCLS + labc
    keep2 = _last_wins_kept(list(key2), valid)

    tbox = np.zeros((128, 4), np.float32)
    tbox[:, 0] = np.where(valid, cx / f32(4.0) - gxf - f32(0.5), f32(0.0))
    tbox[:, 1] = np.where(valid, cy / f32(4.0) - gyf - f32(0.5), f32(0.0))
    tbox[:, 2] = np.where(valid, np.log(np.maximum(bw * f32(0.25) + f32(1e-6), f32(1e-20))).astype(f32), f32(0.0))
    tbox[:, 3] = np.where(valid, np.log(np.maximum(bh * f32(0.25) + f32(1e-6), f32(1e-20))).astype(f32), f32(0.0))
    onehot = (labc[:, None] == np.arange(NCLS)[None, :]).astype(np.float32)
    parti = np.zeros((128, 8), np.float32)
    parti[:, 2] = kept.astype(np.float32)
    parti[:, 5] = keep2.astype(np.float32)
    return wt128, cellg.reshape(128, 1), tbox, onehot, parti


def _prep(pred_heatmap, pred_boxes, pred_classes, bboxes, labels):
    qg, cvec = _consts()
    pbt = np.ascontiguousarray(
        pred_boxes.transpose(0, 2, 3, 1).reshape(B, PIX, 4)).astype(np.float32)
    pct = np.ascontiguousarray(
        pred_classes.transpose(0, 2, 3, 1).reshape(B, PIX, NCLS)).astype(np.float32)
    # hm2[p, 2 f + img] = hm[img, 128 f + p]
    hmr = np.asarray(pred_heatmap, np.float32).reshape(B, 200, 128)
    bbn = np.asarray(bboxes, np.float32)
    labn = np.asarray(labels)
    in_maps = []
    for c in range(NC):
        s = slice(c * BPC, (c + 1) * BPC)
        hm2 = np.ascontiguousarray(hmr[s].transpose(2, 1, 0).reshape(128, 400))
        wt128, cellg, tbox, onehot, parti = _stage_a(bbn[s], labn[s])
        in_maps.append({
            "qg": qg,
            "wt": wt128,
            "hm2": hm2,
            "pbt": pbt[s].reshape(BPC * PIX, 4),
            "pct": pct[s].reshape(BPC * PIX, NCLS),
            "cellg": cellg, "tboxd": tbox, "onehot": onehot,
            "parti": parti, "cvec": cvec,
        })
    return in_maps


def _combine(pvecs):
    P = np.sum(np.stack(pvecs, 0), axis=0, dtype=np.float64).astype(np.float32)
    heat = P[1] / max(P[0], np.float32(1.0))
    num_pos = max(P[2], np.float32(1.0))
    box = P[3] / num_pos if P[2] > 1.0 else np.float32(0.0)
    cls = (P[4] + P[6]) / max(P[5], np.float32(1.0)) if P[2] > 1.0 else np.float32(0.0)
    return np.float32(heat + box + cls)


def kernel(pred_heatmap, pred_boxes, pred_classes, bboxes, labels):
    if "nc" not in _CACHE:
        _CACHE["nc"] = _build()
    nc = _CACHE["nc"]
    in_maps = _prep(pred_heatmap, pred_boxes, pred_classes, bboxes, labels)
    r = run_bass_kernel_spmd(nc, in_maps, list(range(NC)))
    pvecs = [np.asarray(r.results[c]["out"]).reshape(128, 8).sum(axis=0, dtype=np.float64) for c in range(NC)]
    return _combine(pvecs)


if __name__ == "__main__":
    import reference
    inputs = reference.setup_inputs()
    inputs = {k: np.asarray(v) for k, v in inputs.items()}
    out = kernel(**inputs)
    exp = np.asarray(reference.reference(**inputs))
    rel = abs(out - exp) / max(abs(exp), 1e-9)
    print("expected:", exp, "actual:", out, "rel:", rel)


# revision 28
# speedup vs baseline: 1.0859x; 1.0013x over previous
"""AnchorFreeLoss on 8 TRN2 NeuronCores (v3).

Strategy (data-parallel over batch, 2 images/core):
- Host prep (tiny [B,M] per-object math, like the qgrid/NHWC transposes):
  per-object centers/radii -> packed block-diagonal bf16 coefficient
  matrix wt128 (3-way bf16 split of fp32 coefficients, quadrant-aligned),
  gather offsets, box targets, class one-hots, and scatter dedup flags
  (last valid object per cell wins, matching XLA scatter semantics).
- Device heatmap target: logG[pix, m] = -dist^2/(2*sigma^2) is affine in
  q(pix) = [x^2+y^2, x, y, 1]: ONE bf16 matmul per 512-pixel group
  (K=128 packs 4 pixel-chunks x 15 live rows) into two 4-bank PSUM slabs;
  DVE max-reduces a whole 2048-col slab per instruction. Cutoff dist<=2r
  equals logG >= -8 exactly (sigma = r/2).
- Focal transcendentals on the Scalar engine (incl. per-slab exp of the
  log-heatmap); focal algebra + reductions on DVE.
- Box/class losses only touch object-center cells: GPSIMD indirect-DMA
  gathers from host-transposed [B*H*W, C] tables.
- No collective: GPSIMD cross-lane-reduces the per-partition partial
  sums; each core DMAs out 8 floats; host combines (the unshard step).
"""

import sys
from contextlib import ExitStack

import numpy as np

if "/opt/trn_rl_repo" not in sys.path:
    sys.path.insert(0, "/opt/trn_rl_repo")

import ml_dtypes
from concourse import bass, mybir
from concourse.bass_utils import run_bass_kernel_spmd

F32 = mybir.dt.float32
BF16 = mybir.dt.bfloat16
I32 = mybir.dt.int32
ALU = mybir.AluOpType
ACT = mybir.ActivationFunctionType
AXX = mybir.AxisListType.X

B, M, H, W = 16, 64, 160, 160
NC = 8
BPC = B // NC          # 2 images per core
PIX = H * W            # 25600
NCLS = 43
EPS = 1e-7
THR = -8.0             # log-domain cutoff (= dist <= 2r since sigma = r/2)
NGRP = PIX // 512      # 50 groups of 512 pixels
NSLAB = (NGRP + 3) // 4  # 13 slabs (last one half-size)


def _build(debug=False):
    nc = bass.Bass()

    qg_d = nc.declare_dram_parameter("qg", [60, NGRP * 128], BF16, isOutput=False)
    wt_d = nc.declare_dram_parameter("wt", [60, 512], BF16, isOutput=False)
    hm_d = nc.declare_dram_parameter("hm2", [128, 400], F32, isOutput=False)
    pb_d = nc.declare_dram_parameter("pbt", [BPC * PIX, 4], F32, isOutput=False)
    pc_d = nc.declare_dram_parameter("pct", [BPC * PIX, NCLS], F32, isOutput=False)
    cg_d = nc.declare_dram_parameter("cellg", [128, 1], I32, isOutput=False)
    tb_d = nc.declare_dram_parameter("tboxd", [128, 4], F32, isOutput=False)
    oh_d = nc.declare_dram_parameter("onehot", [128, NCLS], F32, isOutput=False)
    pi_d = nc.declare_dram_parameter("parti", [128, 8], F32, isOutput=False)
    cv_d = nc.declare_dram_parameter("cvec", [128, 8], F32, isOutput=False)
    out_d = nc.declare_dram_parameter("out", [128, 8], F32, isOutput=True)
    dbg = {}
    if debug:
        for nm, shp, dt in [("d_partials", [128, 8], F32),
                            ("d_hmL", [128, 400], F32),
                            ("d_sc", [128, 16], F32),
                            ("d_gb", [128, 4], F32),
                            ("d_gbn", [128, 4], F32),
                            ("d_tbox", [128, 4], F32)]:
            dbg[nm] = nc.declare_dram_parameter(nm, shp, dt, isOutput=True)

    es = ExitStack()
    dS = es.enter_context(nc.semaphore("dS"))        # small input dmas
    dH = es.enter_context(nc.semaphore("dH"))        # hm2
    dQ = es.enter_context(nc.semaphore("dQ"))        # qgrid first chunk
    dQ2 = es.enter_context(nc.semaphore("dQ2"))      # qgrid chunk 2
    dQ3 = es.enter_context(nc.semaphore("dQ3"))      # qgrid chunk 3
    dW = es.enter_context(nc.semaphore("dW"))        # wt128
    dC = es.enter_context(nc.semaphore("dC"))        # cellg
    dO = es.enter_context(nc.semaphore("dO"))        # output
    va = es.enter_context(nc.semaphore("va"))        # vector -> scalar
    av = es.enter_context(nc.semaphore("av"))        # scalar -> vector
    g_s = es.enter_context(nc.semaphore("g_s"))      # gathers done
    pe_s = es.enter_context(nc.semaphore("pe_s"))    # matmul per group
    dv_s = es.enter_context(nc.semaphore("dv_s"))    # slab reduce done
    pt_s = es.enter_context(nc.semaphore("pt_s"))    # partials ready
    pp_s = es.enter_context(nc.semaphore("pp_s"))    # psp matmul done
    pv_s = es.enter_context(nc.semaphore("pv_s"))    # pvec ready

    sQ = es.enter_context(nc.sbuf_tensor("sQ", [60, NGRP * 128], BF16))
    wt128 = es.enter_context(nc.sbuf_tensor("wt128", [60, 512], BF16))
    cvec = es.enter_context(nc.sbuf_tensor("cvec_s", [128, 8], F32))
    sci = es.enter_context(nc.sbuf_tensor("sci", [128, 1], I32))
    hmP = es.enter_context(nc.sbuf_tensor("hmP", [128, 400], F32))
    lnp = es.enter_context(nc.sbuf_tensor("lnp", [128, 400], F32))
    ln1p = es.enter_context(nc.sbuf_tensor("ln1p", [128, 400], F32))
    p2 = es.enter_context(nc.sbuf_tensor("p2", [128, 400], F32))
    q2 = es.enter_context(nc.sbuf_tensor("q2", [128, 400], F32))
    texp = es.enter_context(nc.sbuf_tensor("texp", [128, 400], F32))
    hmL = es.enter_context(nc.sbuf_tensor("hmL", [128, 400], F32))
    tbox = es.enter_context(nc.sbuf_tensor("tbox", [128, 4], F32))
    gb = es.enter_context(nc.sbuf_tensor("gb", [128, 4], F32))
    gbn = es.enter_context(nc.sbuf_tensor("gbn", [128, 4], F32))
    gc = es.enter_context(nc.sbuf_tensor("gc", [128, NCLS], F32))
    gcp = es.enter_context(nc.sbuf_tensor("gcp", [128, NCLS], F32))
    junk43 = es.enter_context(nc.sbuf_tensor("junk43", [128, NCLS], F32))
    jb43 = es.enter_context(nc.sbuf_tensor("jb43", [128, NCLS], F32))
    onehot = es.enter_context(nc.sbuf_tensor("onehot_s", [128, NCLS], F32))
    parti = es.enter_context(nc.sbuf_tensor("parti_s", [128, 8], F32))
    sc = es.enter_context(nc.sbuf_tensor("sc", [128, 16], F32))
    pv = es.enter_context(nc.sbuf_tensor("pv", [1, 8], F32))

    psA = es.enter_context(nc.psum_tensor("psA", [128, 2048], F32))
    psB = es.enter_context(nc.psum_tensor("psB", [128, 2048], F32))

    with es:
        names = ["l1r", "negrow", "plab", "lnpl", "ln1pl", "p2pl", "q2pl",
                 "cva", "sent"]
        col = {n: sc[:, i: i + 1] for i, n in enumerate(names)}
        nc.const_aps.aps[(F32, 0.0)] = cvec[:, 0:1]
        nc.const_aps.aps[(F32, 1.0)] = cvec[:, 1:2]
        ones = cvec[:, 1:2]
        kept = parti[:, 2:3]
        keep2 = parti[:, 5:6]
        cellg = sci[:, 0:1]

        def slab_ngroups(k):
            return min(4, NGRP - 4 * k)

        def slab_in(k):
            pst = psA if k % 2 == 0 else psB
            return pst[:, 0: 512 * slab_ngroups(k)]

        def slab_out(k):
            return hmL[:, 32 * k: 32 * k + 8 * slab_ngroups(k)]

        with nc.Block() as block:

            @block.sync
            def _(sync):
                # critical first: first 8 matmul groups, weights, pred heatmap
                sync.dma_start(out=sQ[:, 0:1024], in_=qg_d[:, 0:1024]).then_inc(dQ, 16)
                sync.dma_start(out=wt128[:, :], in_=wt_d[:, :]).then_inc(dW, 16)
                sync.dma_start(out=hmP[:, :], in_=hm_d[:, :]).then_inc(dH, 16)
                sync.dma_start(out=cvec[:, :], in_=cv_d[:, :]).then_inc(dH, 16)
                sync.dma_start(out=sci[:, :], in_=cg_d[:, :]).then_inc(dC, 16)
                # hold the rest back so the criticals get full DMA bandwidth
                sync.wait_ge(dQ, 16)
                sync.wait_ge(dH, 32)
                sync.dma_start(out=parti[:, :], in_=pi_d[:, :]).then_inc(dS, 16)
                sync.dma_start(out=tbox[:, :], in_=tb_d[:, :]).then_inc(dS, 16)
                sync.dma_start(out=sQ[:, 1024:3072], in_=qg_d[:, 1024:3072]).then_inc(dQ2, 16)
                sync.dma_start(out=sQ[:, 3072:], in_=qg_d[:, 3072:]).then_inc(dQ3, 16)
                sync.dma_start(out=onehot[:, :], in_=oh_d[:, :]).then_inc(dS, 16)
                sync.wait_ge(pt_s, 1)
                sync.dma_start(out=out_d[:, :], in_=parti[:, :]).then_inc(dO, 16)
                ndO = 16
                if debug:
                    for nm, t in [("d_partials", parti), ("d_hmL", hmL),
                                  ("d_sc", sc), ("d_gb", gb),
                                  ("d_gbn", gbn), ("d_tbox", tbox)]:
                        sync.dma_start(out=dbg[nm][:, :], in_=t[:, :]).then_inc(dO, 16)
                        ndO += 16
                sync.wait_ge(dO, ndO)

            @block.gpsimd
            def _(gpsimd):
                gpsimd.wait_ge(dC, 16)
                gpsimd.indirect_dma_start(
                    out=gb[:, :], out_offset=None,
                    in_=pb_d[:, :],
                    in_offset=bass.IndirectOffsetOnAxis(ap=cellg, axis=0),
                ).then_inc(g_s, 16)
                gpsimd.indirect_dma_start(
                    out=gc[:, :], out_offset=None,
                    in_=pc_d[:, :],
                    in_offset=bass.IndirectOffsetOnAxis(ap=cellg, axis=0),
                ).then_inc(g_s, 16)


            @block.tensor
            def _(tensor):
                tensor.wait_ge(dW, 16)
                tensor.wait_ge(dQ, 16)
                for g in range(NGRP):
                    if g == 8:
                        tensor.wait_ge(dQ2, 16)
                    if g == 24:
                        tensor.wait_ge(dQ3, 16)
                    pst = psA if (g // 4) % 2 == 0 else psB
                    if g >= 8:
                        tensor.wait_ge(dv_s, g // 4 - 1)
                    tensor.matmul(
                        pst[:, 512 * (g % 4): 512 * (g % 4 + 1)],
                        sQ[:, g * 128: (g + 1) * 128],
                        wt128[:, :],
                        start=True, stop=True, skip_group_check=True,
                    ).then_inc(pe_s, 1)

            @block.scalar
            def _(scalar):
                # pred-heatmap transcendentals (read clipped hmP only)
                scalar.wait_ge(va, 1)
                scalar.activation(lnp[:, :], hmP[:, :], ACT.Ln)
                scalar.activation(ln1p[:, :], hmP[:, :], ACT.Ln, bias=1.0, scale=-1.0)
                scalar.activation(p2[:, :], hmP[:, :], ACT.Square)
                scalar.activation(q2[:, :], hmP[:, :], ACT.Square, bias=1.0, scale=-1.0)
                scalar.drain().then_inc(av, 1)                      # av1
                # cls: p = sigmoid(x), softplus(x) = -ln(1-p), p^2 via tables
                scalar.wait_ge(g_s, 32)
                scalar.activation(gcp[:, :], gc[:, :], ACT.Sigmoid)
                scalar.activation(junk43[:, :], gcp[:, :], ACT.Ln, bias=1.0, scale=-1.0)
                scalar.activation(gc[:, :], gcp[:, :], ACT.Square)
                scalar.drain().then_inc(av, 1)                      # av2
                scalar.wait_ge(va, 2)
                scalar.activation(col["lnpl"], col["plab"], ACT.Ln)
                scalar.activation(col["ln1pl"], col["plab"], ACT.Ln, bias=1.0, scale=-1.0)
                scalar.activation(col["p2pl"], col["plab"], ACT.Square)
                scalar.activation(col["q2pl"], col["plab"], ACT.Square, bias=1.0, scale=-1.0)
                scalar.drain().then_inc(av, 1)                      # av3
                # per-slab exp of the log-heatmap target
                for k in range(NSLAB):
                    scalar.wait_ge(dv_s, min(k + 2, NSLAB))
                    n8 = 8 * slab_ngroups(k)
                    scalar.activation(texp[:, 32 * k: 32 * k + n8],
                                      hmL[:, 32 * k: 32 * k + n8], ACT.Exp)
                scalar.drain().then_inc(av, 1)                      # av4

            @block.vector
            def _(v):
                ts, stt = v.tensor_scalar, v.scalar_tensor_tensor

                def slot(*thunks):
                    for t in thunks:
                        t()
                    v.drain()

                # heatmap slab reduces + interleaved one-shot work
                for k in range(NSLAB):
                    if k == 0:
                        v.wait_ge(pe_s, 2)
                        v.tensor_reduce(
                            out=hmL[:, 0:16],
                            in_=psA[:, 0:1024].rearrange("p (G m) -> p G m", G=16),
                            op=ALU.max, axis=AXX)
                        v.wait_ge(pe_s, 4)
                        v.tensor_reduce(
                            out=hmL[:, 16:32],
                            in_=psA[:, 1024:2048].rearrange("p (G m) -> p G m", G=16),
                            op=ALU.max, axis=AXX,
                        ).then_inc(dv_s, 1)
                        continue
                    v.wait_ge(pe_s, min(4 * k + 4, NGRP))
                    v.tensor_reduce(
                        out=slab_out(k),
                        in_=slab_in(k).rearrange("p (G m) -> p G m",
                                                 G=8 * slab_ngroups(k)),
                        op=ALU.max, axis=AXX,
                    ).then_inc(dv_s, 1)
                    if k == 3:
                        # clip predicted heatmap + box l1 (independent, share slots)
                        v.wait_ge(dH, 32)
                        v.wait_ge(g_s, 32)
                        v.wait_ge(dS, 32)
                        slot(lambda: ts(hmP[:, :], hmP[:, :], EPS, 1.0 - EPS, op0=ALU.max, op1=ALU.min),
                             lambda: v.tensor_sub(gbn[:, :], gb[:, :], tbox[:, :]))
                        v.sem_inc(va, 1)                            # va1
                        slot(lambda: stt(gbn[:, :], gbn[:, :], -1.0, gbn[:, :], op0=ALU.mult, op1=ALU.max))
                        slot(lambda: v.tensor_reduce(out=col["l1r"], in_=gbn[:, :], op=ALU.add, axis=AXX))
                        slot(lambda: v.tensor_mul(parti[:, 3:4], col["l1r"], kept))
                    if k == 5:
                        # focal A/B coefficient maps
                        v.wait_ge(av, 1)
                        slot(lambda: stt(lnp[:, :], q2[:, :], -0.25, lnp[:, :], op0=ALU.mult, op1=ALU.mult),
                             lambda: stt(ln1p[:, :], p2[:, :], 0.75, ln1p[:, :], op0=ALU.mult, op1=ALU.mult))
                    if k == 6:
                        v.wait_ge(av, 2)
                        slot(lambda: stt(junk43[:, :], gc[:, :], -0.75, junk43[:, :],
                                         op0=ALU.mult, op1=ALU.mult, accum_out=col["negrow"]),
                             lambda: v.tensor_mul(jb43[:, :], gcp[:, :], onehot[:, :]))
                        slot(lambda: v.tensor_mul(parti[:, 4:5], col["negrow"], kept),
                             lambda: v.tensor_reduce(out=col["plab"], in_=jb43[:, :], op=ALU.add, axis=AXX))
                        v.sem_inc(va, 1)                            # va2 (plab ready)
                    if k == 8:
                        v.wait_ge(av, 3)
                        slot(lambda: stt(col["cva"], col["q2pl"], -0.25, col["lnpl"], op0=ALU.mult, op1=ALU.mult),
                             lambda: stt(col["sent"], col["p2pl"], -0.75, col["ln1pl"], op0=ALU.mult, op1=ALU.mult))
                        slot(lambda: v.tensor_sub(col["cva"], col["cva"], col["sent"]))
                        slot(lambda: v.tensor_mul(parti[:, 6:7], col["cva"], keep2))
                # ---- heat focal tail (cutoff mask dropped: exp(logG) <= e^-8
                # below threshold, which perturbs only (1-t) by <=3.4e-4) ----
                v.drain()
                v.wait_ge(av, 4)
                slot(lambda: v.tensor_mul(lnp[:, :], lnp[:, :], texp[:, :]),
                     lambda: stt(ln1p[:, :], texp[:, :], 1.0, ln1p[:, :], op0=ALU.subtract, op1=ALU.mult))
                slot(lambda: ts(q2[:, :], texp[:, :], 0.5, None, op0=ALU.is_gt))
                slot(lambda: v.tensor_reduce(out=parti[:, 0:1], in_=q2[:, :], op=ALU.add, axis=AXX),
                     lambda: v.tensor_sub(lnp[:, :], lnp[:, :], ln1p[:, :]))
                slot(lambda: v.tensor_mul(lnp[:, :], lnp[:, :], q2[:, :]))
                slot(lambda: stt(lnp[:, :], lnp[:, :], 1.0, ln1p[:, :],
                                 op0=ALU.mult, op1=ALU.add, accum_out=parti[:, 1:2]))
                v.sem_inc(pt_s, 1)

    return nc


_CACHE = {}


def _consts():
    j = np.arange(PIX)
    x = (j % W).astype(np.float32)
    y = (j // W).astype(np.float32)
    q1 = x * x + y * y
    q1hi = q1.astype(ml_dtypes.bfloat16).astype(np.float32)
    q1lo = q1 - q1hi
    onesv = np.ones_like(x)
    q5 = np.stack([q1hi, q1lo, x, y, onesv])           # [5, PIX] all bf16-exact
    # qg[15 c + 5 s + r, 128 g + p] = q5[r, 512 g + 128 c + p]
    q5r = q5.reshape(5, NGRP, 4, 128)                  # [r, g, c, p]
    qg = np.zeros((60, NGRP * 128), np.float32)
    for c in range(4):
        for s in range(3):
            qg[15 * c + 5 * s: 15 * c + 5 * s + 5, :] = (
                q5r[:, :, c, :].reshape(5, NGRP * 128))
    qg = qg.astype(ml_dtypes.bfloat16)
    cvec = np.zeros((128, 8), dtype=np.float32)
    cvec[:, 1] = 1.0
    return qg, cvec


def _last_wins_kept(keys, valid):
    """kept[i] = valid[i] and no valid j>i with keys[j]==keys[i]."""
    n = len(keys)
    kept = np.zeros(n, bool)
    seen = set()
    for i in range(n - 1, -1, -1):
        if valid[i] and keys[i] not in seen:
            kept[i] = True
            seen.add(keys[i])
    return kept


def _stage_a(bboxes, labels):
    """Per-core-chunk object prep: returns wt128, cellg, tbox, onehot, parti."""
    f32 = np.float32
    bb = bboxes.reshape(128, 4).astype(f32)
    lab = labels.reshape(128).astype(np.int64)
    x1, y1, x2, y2 = bb[:, 0], bb[:, 1], bb[:, 2], bb[:, 3]
    cx = (x1 + x2) / f32(2.0)
    cy = (y1 + y2) / f32(2.0)
    bw = x2 - x1
    bh = y2 - y1
    valid = (lab >= 0) & (bb.sum(1) > 0) & (bw > 0) & (bh > 0)
    gx = np.clip((cx / f32(4.0)).astype(np.int32), 0, W - 1)
    gy = np.clip((cy / f32(4.0)).astype(np.int32), 0, H - 1)
    r = np.maximum(np.sqrt(np.maximum(bw * bh, f32(0.0))) / f32(4.0), f32(2.0)).astype(np.int32).astype(f32)
    nscv = f32(-2.0) / (r * r)
    gxf = gx.astype(f32)
    gyf = gy.astype(f32)
    w1 = np.where(valid, nscv, f32(0))
    w2 = np.where(valid, f32(-2.0) * nscv * gxf, f32(0))
    w3 = np.where(valid, f32(-2.0) * nscv * gyf, f32(0))
    w4 = np.where(valid, nscv * (gxf * gxf + gyf * gyf), f32(-1e30))
    Wm = np.stack([w1, w1, w2, w3, w4]).astype(f32)    # [5, 128]
    # 3-way bf16 split
    a_ = Wm.astype(ml_dtypes.bfloat16).astype(f32)
    r1_ = Wm - a_
    b_ = r1_.astype(ml_dtypes.bfloat16).astype(f32)
    r2_ = r1_ - b_
    c_ = r2_.astype(ml_dtypes.bfloat16).astype(f32)
    w15 = np.concatenate([a_, b_, c_], axis=0)         # [15, 128]
    wt128 = np.zeros((60, 512), np.float32)
    for c in range(4):
        wt128[15 * c: 15 * c + 15, 128 * c: 128 * (c + 1)] = w15
    wt128 = wt128.astype(ml_dtypes.bfloat16)

    img = np.arange(128) // M
    cell = gy.astype(np.int64) * W + gx.astype(np.int64)
    cellg = (cell + img * PIX).astype(np.int32)
    kept = _last_wins_kept(list(cellg), valid)
    labc = np.clip(lab, 0, NCLS - 1)
    key2 = cellg.astype(np.int64) * NCLS + labc
    keep2 = _last_wins_kept(list(key2), valid)

    tbox = np.zeros((128, 4), np.float32)
    tbox[:, 0] = np.where(valid, cx / f32(4.0) - gxf - f32(0.5), f32(0.0))
    tbox[:, 1] = np.where(valid, cy / f32(4.0) - gyf - f32(0.5), f32(0.0))
    tbox[:, 2] = np.where(valid, np.log(np.maximum(bw * f32(0.25) + f32(1e-6), f32(1e-20))).astype(f32), f32(0.0))
    tbox[:, 3] = np.where(valid, np.log(np.maximum(bh * f32(0.25) + f32(1e-6), f32(1e-20))).astype(f32), f32(0.0))
    onehot = (labc[:, None] == np.arange(NCLS)[None, :]).astype(np.float32)
    parti = np.zeros((128, 8), np.float32)
    parti[:, 2] = kept.astype(np.float32)
    parti[:, 5] = keep2.astype(np.float32)
    return wt128, cellg.reshape(128, 1), tbox, onehot, parti


def _prep(pred_heatmap, pred_boxes, pred_classes, bboxes, labels):
    qg, cvec = _consts()
    pbt = np.ascontiguousarray(
        pred_boxes.transpose(0, 2, 3, 1).reshape(B, PIX, 4)).astype(np.float32)
    pct = np.ascontiguousarray(
        pred_classes.transpose(0, 2, 3, 1).reshape(B, PIX, NCLS)).astype(np.float32)
    # hm2[p, 2 f + img] = hm[img, 128 f + p]
    hmr = np.asarray(pred_heatmap, np.float32).reshape(B, 200, 128)
    bbn = np.asarray(bboxes, np.float32)
    labn = np.asarray(labels)
    in_maps = []
    for c in range(NC):
        s = slice(c * BPC, (c + 1) * BPC)
        hm2 = np.ascontiguousarray(hmr[s].transpose(2, 1, 0).reshape(128, 400))
        wt128, cellg, tbox, onehot, parti = _stage_a(bbn[s], labn[s])
        in_maps.append({
            "qg": qg,
            "wt": wt128,
            "hm2": hm2,
            "pbt": pbt[s].reshape(BPC * PIX, 4),
            "pct": pct[s].reshape(BPC * PIX, NCLS),
            "cellg": cellg, "tboxd": tbox, "onehot": onehot,
            "parti": parti, "cvec": cvec,
        })
    return in_maps


def _combine(pvecs):
    P = np.sum(np.stack(pvecs, 0), axis=0, dtype=np.float64).astype(np.float32)
    heat = P[1] / max(P[0], np.float32(1.0))
    num_pos = max(P[2], np.float32(1.0))
    box = P[3] / num_pos if P[2] > 1.0 else np.float32(0.0)
    cls = (P[4] + P[6]) / max(P[5], np.float32(1.0)) if P[2] > 1.0 else np.float32(0.0)
    return np.float32(heat + box + cls)


def kernel(pred_heatmap, pred_boxes, pred_classes, bboxes, labels):
    if "nc" not in _CACHE:
        _CACHE["nc"] = _build()
    nc = _CACHE["nc"]
    in_maps = _prep(pred_heatmap, pred_boxes, pred_classes, bboxes, labels)
    r = run_bass_kernel_spmd(nc, in_maps, list(range(NC)))
    pvecs = [np.asarray(r.results[c]["out"]).reshape(128, 8).sum(axis=0, dtype=np.float64) for c in range(NC)]
    return _combine(pvecs)


if __name__ == "__main__":
    import reference
    inputs = reference.setup_inputs()
    inputs = {k: np.asarray(v) for k, v in inputs.items()}
    out = kernel(**inputs)
    exp = np.asarray(reference.reference(**inputs))
    rel = abs(out - exp) / max(abs(exp), 1e-9)
    print("expected:", exp, "actual:", out, "rel:", rel)
